# revision 1
# baseline (speedup 1.0000x reference)
"""
nn_MBPerformerEEG kernel for 8 trn2 NeuronCores (axon/PJRT backend).

Sharding: data-parallel over batch (B=16 -> 2 per core). The (faithful)
KV einsum sums over batch AND sequence, so the per-layer local KV
[H,F,D] partials (8x2KB) are summed on host between device phases --
the only cross-core communication. Everything else (tokenizer,
layernorms, FAVOR features, attention apply, MLP, attention-pool head)
is batch-parallel on-device. Heavy intermediates (h, Q, V) stay
device-resident between phases; weights are replicated to all 8 cores
once and cached across calls.
"""
import numpy as np
import jax
import jax.numpy as jnp

B, E, T = 16, 64, 1000
DIM, HEADS, NB_F, LAYERS, NUM_CLASSES = 32, 4, 64, 2, 2
BAND_K = 51
DH = DIM // HEADS
EPS = 1e-5
NCORES = 8
BPC = B // NCORES


def _layernorm(x, g, b):
    mu = jnp.mean(x, axis=-1, keepdims=True)
    var = jnp.var(x, axis=-1, keepdims=True)
    return (x - mu) * jax.lax.rsqrt(var + EPS) * g + b


def _favor(x, pm):
    b, n, c = x.shape
    xh = x.reshape(b, n, HEADS, DH)
    p = jnp.einsum('bnhd,hdf->bnhf', xh, pm)
    return jax.nn.elu(p) + 1.0


def _tokenize(x, band_w, pw_w, bn_g, bn_b, bn_m, bn_v):
    pad = BAND_K // 2
    xc = jax.lax.conv_general_dilated(
        x, band_w, window_strides=(1, 1), padding=((0, 0), (pad, pad)),
        dimension_numbers=('NCHW', 'OIHW', 'NCHW'))
    h = jnp.einsum('bcet,oc->boet', xc, pw_w)
    h = (h - bn_m[:, None, None]) * jax.lax.rsqrt(bn_v[:, None, None] + EPS) \
        * bn_g[:, None, None] + bn_b[:, None, None]
    h = jax.nn.gelu(h, approximate=False)
    b, c, e, t = h.shape
    h = h.reshape(b, c, e, t // 4, 4).mean(axis=-1)
    h = h.reshape(b, c, e * (t // 4))
    return jnp.transpose(h, (0, 2, 1))


def _pre_kv(h, ln1_g, ln1_b, wqkv, pm):
    xl = _layernorm(h, ln1_g, ln1_b)
    qkv = xl @ wqkv
    q, k, v = jnp.split(qkv, 3, axis=-1)
    Q = _favor(q, pm)
    K = _favor(k, pm)
    V = v.reshape(v.shape[0], v.shape[1], HEADS, DH)
    KV_local = jnp.einsum('bnhf,bnhd->hfd', K, V)
    return Q, V, KV_local


def _post_kv(h, Q, KV, wproj, bproj, ln2_g, ln2_b, w1, b1, w2, b2):
    Z = 1.0 / jnp.einsum('bnhf,hfd->bnh', Q, KV)
    out = jnp.einsum('bnhf,hfd->bnhd', Q, KV) * Z[..., None]
    b_, n_, _, _ = out.shape
    h = h + out.reshape(b_, n_, DIM) @ wproj + bproj
    m = _layernorm(h, ln2_g, ln2_b)
    m = jax.nn.gelu(m @ w1 + b1, approximate=False) @ w2 + b2
    return h + m


def _phase1(x, band_w, pw_w, bn_g, bn_b, bn_m, bn_v, ln1_g, ln1_b, wqkv, pm):
    h = _tokenize(x, band_w, pw_w, bn_g, bn_b, bn_m, bn_v)
    Q, V, KV_local = _pre_kv(h, ln1_g, ln1_b, wqkv, pm)
    return h, Q, V, KV_local


def _phase2(h, Q, V, KV0, wproj, bproj, ln2_g, ln2_b, w1, b1, w2, b2,
            ln1_g1, ln1_b1, wqkv1, pm1):
    h = _post_kv(h, Q, KV0, wproj, bproj, ln2_g, ln2_b, w1, b1, w2, b2)
    Q1, V1, KV1_local = _pre_kv(h, ln1_g1, ln1_b1, wqkv1, pm1)
    return h, Q1, V1, KV1_local


def _phase3(h, Q1, V1, KV1, wproj, bproj, ln2_g, ln2_b, w1, b1, w2, b2,
            norm_g, norm_b, pool_w, pool_b, fc_w, fc_b):
    h = _post_kv(h, Q1, KV1, wproj, bproj, ln2_g, ln2_b, w1, b1, w2, b2)
    h = _layernorm(h, norm_g, norm_b)
    w = jax.nn.softmax(h @ pool_w + pool_b, axis=1)
    pooled = jnp.sum(w * h, axis=1)
    return pooled @ fc_w + fc_b


_STATE = {}


def _build(weights):
    """Compile pmaps and replicate weights to all devices once."""
    if 'P1' in _STATE:
        return
    devs = jax.devices()[:NCORES]
    rep = lambda a: jax.device_put_replicated(jnp.asarray(a, jnp.float32), devs)
    _STATE['w'] = {k: rep(v) for k, v in weights.items()}
    _STATE['P1'] = jax.pmap(_phase1)                 # all args sharded (weights pre-replicated)
    _STATE['P2'] = jax.pmap(_phase2)
    _STATE['P3'] = jax.pmap(_phase3)


def kernel(x, band_w, pw_w, bn_g, bn_b, bn_m, bn_v, ln1_g, ln1_b, wqkv,
           proj_mat, wproj, bproj, ln2_g, ln2_b, w1, b1, w2, b2,
           norm_g, norm_b, pool_w, pool_b, fc_w, fc_b):
    _build(dict(
        band_w=band_w, pw_w=pw_w, bn_g=bn_g, bn_b=bn_b, bn_m=bn_m, bn_v=bn_v,
        ln1_g0=ln1_g[0], ln1_b0=ln1_b[0], wqkv0=wqkv[0], pm0=proj_mat[0],
        ln1_g1=ln1_g[1], ln1_b1=ln1_b[1], wqkv1=wqkv[1], pm1=proj_mat[1],
        wproj0=wproj[0], bproj0=bproj[0], ln2_g0=ln2_g[0], ln2_b0=ln2_b[0],
        w10=w1[0], b10=b1[0], w20=w2[0], b20=b2[0],
        wproj1=wproj[1], bproj1=bproj[1], ln2_g1=ln2_g[1], ln2_b1=ln2_b[1],
        w11=w1[1], b11=b1[1], w21=w2[1], b21=b2[1],
        norm_g=norm_g, norm_b=norm_b, pool_w=pool_w, pool_b=pool_b,
        fc_w=fc_w, fc_b=fc_b))
    W = _STATE['w']
    devs = jax.devices()[:NCORES]
    xs = jnp.asarray(np.asarray(x, np.float32).reshape(NCORES, BPC, 1, E, T))

    h, Q, V, KV0l = _STATE['P1'](xs, W['band_w'], W['pw_w'], W['bn_g'],
                                 W['bn_b'], W['bn_m'], W['bn_v'],
                                 W['ln1_g0'], W['ln1_b0'], W['wqkv0'], W['pm0'])
    KV0 = np.asarray(KV0l).sum(axis=0)  # 8x2KB host all-reduce
    KV0r = jax.device_put_replicated(jnp.asarray(KV0), devs)

    h, Q1, V1, KV1l = _STATE['P2'](h, Q, V, KV0r, W['wproj0'], W['bproj0'],
                                   W['ln2_g0'], W['ln2_b0'], W['w10'],
                                   W['b10'], W['w20'], W['b20'],
                                   W['ln1_g1'], W['ln1_b1'], W['wqkv1'],
                                   W['pm1'])
    KV1 = np.asarray(KV1l).sum(axis=0)
    KV1r = jax.device_put_replicated(jnp.asarray(KV1), devs)

    out = _STATE['P3'](h, Q1, V1, KV1r, W['wproj1'], W['bproj1'],
                       W['ln2_g1'], W['ln2_b1'], W['w11'], W['b11'],
                       W['w21'], W['b21'], W['norm_g'], W['norm_b'],
                       W['pool_w'], W['pool_b'], W['fc_w'], W['fc_b'])
    return np.asarray(out).reshape(B, NUM_CLASSES)
